# revision 17
# baseline (speedup 1.0000x reference)
"""MoE (top-2 routing, 8 experts, SwiGLU) on 8 Trainium2 NeuronCores.

Strategy (expert-parallel, per sharding hint):
  - Host computes the router (tiny [T,E] matmul), top-2 + softmax, and
    dispatches tokens to their experts ("all-to-all" = host-side shard).
  - Core e holds expert e's full w1/w3/w2 and runs a dense SwiGLU MLP over
    the tokens routed to it (padded to a uniform capacity C so all 8 cores
    run one SPMD program).
  - Device layout is "transposed activations": tokens live on the free
    dim, feature dims on partitions, so no on-device transposes are needed
    anywhere. Host pre-permutes x / w1 / w3 / w2 into the exact SBUF tile
    consumption order so every DMA is contiguous per partition.
  - Matmuls run as float32r (full PE rate for moving dim >= 256, fp32
    storage, ~tf32 mantissa); accumulation is fp32 in PSUM.
  - The combine weight (softmax gate) is applied on device by multiplying
    the output tile with a host-broadcast [128, C] combine plane.
  - Host scatter-adds each expert's scaled output back into [T, D].
"""

import sys

import numpy as np

sys.path.insert(0, "/opt/trn_rl_repo")

P = 128          # partitions
D = 1024         # model dim
H = 2816         # expert hidden dim
E = 8            # experts == cores
TOPK = 2
CW = 768         # token chunk width (free dim) per pipeline stage
ND, NH = D // P, H // P  # 8, 22

_CACHE: dict[int, object] = {}
LAST_RESULT = None  # BassKernelResults of the most recent run (for test.py)
TRACE = False
BODY_REPEATS = 1  # >1 bakes R identical passes into the program (timing only)
MM_ONLY = False   # timing probe: emit only matmuls+input DMAs
MM_DTYPE = "f32r"  # "f32r" (~tf32 accuracy) or "bf16" (~3.5% faster, 16x worse err)


def _mm_slices(w):
    """Split a chunk width into matmul moving-dim slices (<=512 each)."""
    out, o = [], 0
    while w - o > 512:
        out.append((o, 512))
        o += 512
    out.append((o, w - o))
    return out


def _build(C):
    """Build + compile the per-core SPMD Bass program for capacity C."""
    import concourse.bacc as bacc
    import concourse.bass as bass
    import concourse.mybir as mybir
    import concourse.tile as tile

    f32 = mybir.dt.float32
    f32r = mybir.dt.float32r if MM_DTYPE == "f32r" else mybir.dt.bfloat16
    ACT = mybir.ActivationFunctionType

    nchunks = C // CW
    sl = _mm_slices(CW)

    nc = bacc.Bacc(None, target_bir_lowering=False)
    # All pre-permuted on host so each DMA is contiguous per partition:
    #   xt[c][p][dk][cw]  <- x.T tiles       (4KB runs per partition)
    #   w1[hm][p][dk][h]  <- w1 col-blocks   (4KB runs)
    #   w3[hm][p][dk][h]  <- w3 col-blocks   (4KB runs)
    #   w2[dm][p][hk][d]  <- w2 col-blocks   (11KB runs)
    xt_d = nc.dram_tensor("xt", [nchunks, P, ND, CW], f32r, kind="ExternalInput")
    w1_d = nc.dram_tensor("w1", [NH, P, ND, P], f32r, kind="ExternalInput")
    w3_d = nc.dram_tensor("w3", [NH, P, ND, P], f32r, kind="ExternalInput")
    w2_d = nc.dram_tensor("w2", [ND, P, NH, P], f32r, kind="ExternalInput")
    cb_d = nc.dram_tensor("cb", [P, C], f32, kind="ExternalInput")
    yt_d = nc.dram_tensor("yt", [D, C], f32, kind="ExternalOutput")

    with tile.TileContext(nc) as tc:
        with (
            tc.tile_pool(name="big", bufs=1) as big,
            tc.tile_pool(name="io", bufs=2) as io,
            tc.tile_pool(name="wts", bufs=3) as wts,
            tc.tile_pool(name="psh", bufs=1, space=bass.MemorySpace.PSUM) as psh,
            tc.tile_pool(name="psy", bufs=2, space=bass.MemorySpace.PSUM) as psy,
        ):

            def emit_chunk(c):
                c0 = c * CW
                xt = io.tile([P, ND, CW], f32r, tag="xt")
                nc.sync.dma_start(xt[:], xt_d[c])
                cb = io.tile([P, CW], f32, tag="cb")
                nc.sync.dma_start(cb[:], cb_d[:, c0 : c0 + CW])
                gts = []

                # ---- phase 1: g = silu(x @ w1) * (x @ w3), transposed ----
                for hm in range(NH):
                    w1t = wts.tile([P, ND, P], f32r, tag="w1")
                    nc.sync.dma_start(w1t[:], w1_d[hm])
                    w3t = wts.tile([P, ND, P], f32r, tag="w3")
                    nc.sync.dma_start(w3t[:], w3_d[hm])
                    ph1 = psh.tile([P, CW], f32, tag="h1")
                    ph3 = psh.tile([P, CW], f32, tag="h3")
                    for dk in range(ND):
                        for s0, sw in sl:
                            nc.tensor.matmul(
                                ph1[:, s0 : s0 + sw],
                                w1t[:, dk, :],
                                xt[:, dk, s0 : s0 + sw],
                                start=(dk == 0),
                                stop=(dk == ND - 1),
                            )
                    if not MM_ONLY:
                        s1 = io.tile([P, CW], f32, tag="s1")
                        nc.scalar.activation(s1[:], ph1[:], ACT.Sigmoid)
                        t1 = io.tile([P, CW], f32, tag="t1")
                        nc.vector.tensor_mul(t1[:], s1[:], ph1[:])  # silu(h1)
                    for dk in range(ND):
                        for s0, sw in sl:
                            nc.tensor.matmul(
                                ph3[:, s0 : s0 + sw],
                                w3t[:, dk, :],
                                xt[:, dk, s0 : s0 + sw],
                                start=(dk == 0),
                                stop=(dk == ND - 1),
                            )
                    g = big.tile([P, CW], f32r, tag=f"g{hm}")
                    if not MM_ONLY:
                        nc.vector.tensor_mul(g[:], t1[:], ph3[:])
                    else:
                        nc.sync.dma_start(g[:], xt_d[c][:, 0, :CW])
                    gts.append(g)

                # ---- phase 2: yT = (g @ w2).T * combine ----
                for dm in range(ND):
                    d0 = dm * P
                    w2t = wts.tile([P, NH, P], f32r, tag="w2", bufs=2)
                    nc.sync.dma_start(w2t[:], w2_d[dm])
                    py = psy.tile([P, CW], f32, tag="y")
                    for hk in range(NH):
                        for s0, sw in sl:
                            nc.tensor.matmul(
                                py[:, s0 : s0 + sw],
                                w2t[:, hk, :],
                                gts[hk][:, s0 : s0 + sw],
                                start=(hk == 0),
                                stop=(hk == NH - 1),
                            )
                    if not MM_ONLY or dm == 0:
                        y = io.tile([P, CW], f32, tag="y_out")
                        nc.vector.tensor_mul(y[:], py[:], cb[:])
                        nc.sync.dma_start(yt_d[d0 : d0 + P, c0 : c0 + CW], y[:])

            if BODY_REPEATS > 1:
                with tc.For_i(0, BODY_REPEATS, 1):
                    for c in range(nchunks):
                        emit_chunk(c)
            else:
                for c in range(nchunks):
                    emit_chunk(c)

    nc.compile()
    return nc


def _permute_inputs(xf_pad, w1e, w3e, w2e, C):
    """Pre-permute per-core tensors into the kernel's DMA-contiguous layouts."""
    nchunks = C // CW
    # xt: [D, C] -> [nchunks, P, ND, CW]
    xt = (
        xf_pad.reshape(ND, P, nchunks, CW)
        .transpose(2, 1, 0, 3)
        .copy()
    )
    # w1/w3: [D, H] -> [NH, P, ND, P_h]
    w1p = w1e.reshape(ND, P, NH, P).transpose(2, 1, 0, 3).copy()
    w3p = w3e.reshape(ND, P, NH, P).transpose(2, 1, 0, 3).copy()
    # w2: [H, D] -> [ND, P, NH, P_d]
    w2p = w2e.reshape(NH, P, ND, P).transpose(2, 1, 0, 3).copy()
    return xt, w1p, w3p, w2p


def _route(xf, gw):
    """Replicate the reference router in numpy fp32.

    Returns (top_idx [T,2], top_w [T,2]) matching jax.lax.top_k +
    softmax-over-the-2-selected-scores.
    """
    scores = xf @ gw.T                                   # [T, E] fp32
    order = np.argsort(-scores, axis=-1, kind="stable")  # ties -> lower idx
    top_idx = order[:, :TOPK]
    tw = np.take_along_axis(scores, top_idx, axis=-1).astype(np.float32)
    m = tw.max(axis=-1, keepdims=True)
    ex = np.exp(tw - m, dtype=np.float32)
    top_w = ex / ex.sum(axis=-1, keepdims=True)
    return top_idx, top_w


def make_in_maps(x, gate_w, w1, w2, w3):
    """Route + dispatch: returns (in_maps, sel, C, shape)."""
    x = np.asarray(x, dtype=np.float32)
    b, s, d = x.shape
    xf = np.ascontiguousarray(x.reshape(-1, d))
    gw = np.asarray(gate_w, dtype=np.float32)
    w1 = np.asarray(w1, dtype=np.float32)
    w2 = np.asarray(w2, dtype=np.float32)
    w3 = np.asarray(w3, dtype=np.float32)

    top_idx, top_w = _route(xf, gw)

    sel, cwt = [], []
    for ee in range(E):
        hit = top_idx == ee                      # [T, 2] bool
        rows = np.nonzero(hit.any(axis=1))[0]
        sel.append(rows)
        cwt.append(top_w[rows, hit[rows].argmax(axis=1)].astype(np.float32))

    cnt = max(len(r) for r in sel)
    C = ((max(cnt, 1) + CW - 1) // CW) * CW

    in_maps = []
    for ee in range(E):
        rows, wts_e = sel[ee], cwt[ee]
        ne = len(rows)
        xtf = np.zeros((D, C), np.float32)
        xtf[:, :ne] = xf[rows].T
        cb_row = np.zeros((C,), np.float32)
        cb_row[:ne] = wts_e
        xt, w1p, w3p, w2p = _permute_inputs(xtf, w1[ee], w3[ee], w2[ee], C)
        if MM_DTYPE == "bf16":
            import ml_dtypes

            bf = ml_dtypes.bfloat16
            xt, w1p, w3p, w2p = (
                xt.astype(bf),
                w1p.astype(bf),
                w3p.astype(bf),
                w2p.astype(bf),
            )
        in_maps.append(
            {
                "xt": xt,
                "w1": w1p,
                "w3": w3p,
                "w2": w2p,
                "cb": np.ascontiguousarray(np.broadcast_to(cb_row, (P, C))),
            }
        )
    return in_maps, sel, C, (b, s, d)


def _pjrt_fn(nc, in_maps):
    """Build a reusable sharded-jit callable over device-resident inputs."""
    import jax
    from jax.experimental.shard_map import shard_map
    from jax.sharding import Mesh, NamedSharding, PartitionSpec

    from concourse import mybir
    from concourse.bass2jax import (
        _bass_exec_p,
        install_neuronx_cc_hook,
        partition_id_tensor,
    )

    install_neuronx_cc_hook()
    partition_name = nc.partition_id_tensor.name if nc.partition_id_tensor else None
    in_names, out_names, out_avals, zero_outs = [], [], [], []
    for alloc in nc.m.functions[0].allocations:
        if not isinstance(alloc, mybir.MemoryLocationSet):
            continue
        name = alloc.memorylocations[0].name
        if alloc.kind == "ExternalInput":
            if name != partition_name:
                in_names.append(name)
        elif alloc.kind == "ExternalOutput":
            out_names.append(name)
            shape = tuple(alloc.tensor_shape)
            dtype = mybir.dt.np(alloc.dtype)
            out_avals.append(jax.core.ShapedArray(shape, dtype))
            zero_outs.append(np.zeros(shape, dtype))
    n_params = len(in_names)
    all_in = list(in_names) + list(out_names)
    if partition_name is not None:
        all_in.append(partition_name)

    def _body(*args):
        operands = list(args)
        if partition_name is not None:
            operands.append(partition_id_tensor())
        return tuple(
            _bass_exec_p.bind(
                *operands,
                out_avals=tuple(out_avals),
                in_names=tuple(all_in),
                out_names=tuple(out_names),
                lowering_input_output_aliases=(),
                sim_require_finite=True,
                sim_require_nnan=True,
                nc=nc,
            )
        )

    mesh = Mesh(np.asarray(jax.devices()[:E]), ("core",))
    spec = PartitionSpec("core")
    fn = jax.jit(
        shard_map(
            _body,
            mesh=mesh,
            in_specs=(spec,) * (n_params + len(out_names)),
            out_specs=(spec,) * len(out_names),
            check_rep=False,
        ),
        keep_unused=True,
    )
    sh = NamedSharding(mesh, spec)
    dev_in = [
        jax.device_put(
            np.concatenate([np.asarray(in_maps[c][nm]) for c in range(E)], axis=0), sh
        )
        for nm in in_names
    ]
    for z in zero_outs:
        dev_in.append(jax.device_put(np.zeros((E * z.shape[0], *z.shape[1:]), z.dtype), sh))
    for a in dev_in:
        a.block_until_ready()
    return fn, dev_in


def measure_hw_ns(in_maps, C, r_big=41, iters=15):
    """Per-pass HW time via device-side For_i repeat-loop slope."""
    import time as _time

    import jax

    global BODY_REPEATS
    nc1 = _CACHE.get(C)
    if nc1 is None:
        nc1 = _build(C)
        _CACHE[C] = nc1
    old = BODY_REPEATS
    BODY_REPEATS = r_big
    try:
        ncR = _build(C)
    finally:
        BODY_REPEATS = old
    fn1, in1 = _pjrt_fn(nc1, in_maps)
    fnR, inR = _pjrt_fn(ncR, in_maps)
    for _ in range(2):
        jax.block_until_ready(fn1(*in1))
        jax.block_until_ready(fnR(*inR))
    t1s, tRs = [], []
    for _ in range(iters):
        t0 = _time.perf_counter()
        jax.block_until_ready(fn1(*in1))
        t1s.append(_time.perf_counter() - t0)
        t0 = _time.perf_counter()
        jax.block_until_ready(fnR(*inR))
        tRs.append(_time.perf_counter() - t0)
    return (min(tRs) - min(t1s)) * 1e9 / (r_big - 1)


def kernel(x, gate_w, w1, w2, w3):
    global LAST_RESULT
    in_maps, sel, C, (b, s, d) = make_in_maps(x, gate_w, w1, w2, w3)

    nc = _CACHE.get(C)
    if nc is None:
        nc = _build(C)
        _CACHE[C] = nc

    from concourse.bass_utils import run_bass_kernel_spmd

    res = run_bass_kernel_spmd(nc, in_maps, core_ids=list(range(E)), trace=TRACE)
    LAST_RESULT = res

    out = np.zeros((b * s, d), np.float32)
    for ee in range(E):
        ne = len(sel[ee])
        if ne:
            # yt is already combine-scaled on device; rows are unique per
            # expert so fancy-index += is safe.
            out[sel[ee]] += res.results[ee]["yt"][:, :ne].T
    return out.reshape(b, s, d)


# revision 18
# speedup vs baseline: 1.3186x; 1.3186x over previous
"""MoE (top-2 routing, 8 experts, SwiGLU) on 8 Trainium2 NeuronCores.

Strategy (expert-parallel, per sharding hint):
  - Host computes the router (tiny [T,E] matmul), top-2 + softmax, and
    dispatches tokens to their experts ("all-to-all" = host-side shard).
  - Core e holds expert e's full w1/w3/w2 and runs a dense SwiGLU MLP over
    the tokens routed to it (padded to a uniform capacity C so all 8 cores
    run one SPMD program).
  - Device layout is "transposed activations": tokens live on the free
    dim, feature dims on partitions, so no on-device transposes are needed
    anywhere. Host pre-permutes x / w1 / w3 / w2 into the exact SBUF tile
    consumption order so every DMA is contiguous per partition.
  - Matmuls run as float32r (full PE rate for moving dim >= 256, fp32
    storage, ~tf32 mantissa); accumulation is fp32 in PSUM.
  - The combine weight (softmax gate) is applied on device by multiplying
    the output tile with a host-broadcast [128, C] combine plane.
  - Host scatter-adds each expert's scaled output back into [T, D].
"""

import sys

import numpy as np

sys.path.insert(0, "/opt/trn_rl_repo")

P = 128          # partitions
D = 1024         # model dim
H = 2816         # expert hidden dim
E = 8            # experts == cores
TOPK = 2
CGRAN = 256      # capacity granularity (min chunk width; f32r needs N>=256)
CWMAX = 1024     # max chunk width (PSUM: 2 banks per h1/h3/y tensor)
ND, NH = D // P, H // P  # 8, 22


def _chunks(C):
    """Split capacity C (multiple of CGRAN) into chunk widths <= CWMAX."""
    out = []
    while C >= CWMAX:
        out.append(CWMAX)
        C -= CWMAX
    if C:
        out.append(C)
    return out

_CACHE: dict[int, object] = {}
LAST_RESULT = None  # BassKernelResults of the most recent run (for test.py)
TRACE = False
BODY_REPEATS = 1  # >1 bakes R identical passes into the program (timing only)
MM_ONLY = False   # timing probe: emit only matmuls+input DMAs
MM_DTYPE = "f32r"  # "f32r" (~tf32 accuracy) or "bf16" (~3.5% faster, 16x worse err)


def _mm_slices(w):
    """Split a chunk width into matmul moving-dim slices (<=512 each)."""
    out, o = [], 0
    while w - o > 512:
        out.append((o, 512))
        o += 512
    out.append((o, w - o))
    return out


def _build(C):
    """Build + compile the per-core SPMD Bass program for capacity C."""
    import concourse.bacc as bacc
    import concourse.bass as bass
    import concourse.mybir as mybir
    import concourse.tile as tile

    f32 = mybir.dt.float32
    f32r = mybir.dt.float32r if MM_DTYPE == "f32r" else mybir.dt.bfloat16
    ACT = mybir.ActivationFunctionType

    chunks = _chunks(C)

    nc = bacc.Bacc(None, target_bir_lowering=False)
    # All pre-permuted on host so each DMA is contiguous per partition:
    #   xt[c][p][dk][cw]  <- x.T tiles       (4KB runs per partition)
    #   w1[hm][p][dk][h]  <- w1 col-blocks   (4KB runs)
    #   w3[hm][p][dk][h]  <- w3 col-blocks   (4KB runs)
    #   w2[dm][p][hk][d]  <- w2 col-blocks   (11KB runs)
    xt_d = nc.dram_tensor("xt", [P, ND, C], f32r, kind="ExternalInput")
    w1_d = nc.dram_tensor("w1", [NH, P, ND, P], f32r, kind="ExternalInput")
    w3_d = nc.dram_tensor("w3", [NH, P, ND, P], f32r, kind="ExternalInput")
    w2_d = nc.dram_tensor("w2", [ND, P, NH, P], f32r, kind="ExternalInput")
    cb_d = nc.dram_tensor("cb", [P, C], f32, kind="ExternalInput")
    yt_d = nc.dram_tensor("yt", [D, C], f32, kind="ExternalOutput")

    with tile.TileContext(nc) as tc:
        with (
            tc.tile_pool(name="big", bufs=1) as big,
            tc.tile_pool(name="io", bufs=2) as io,
            tc.tile_pool(name="wts", bufs=3) as wts,
            tc.tile_pool(name="psh", bufs=1, space=bass.MemorySpace.PSUM) as psh,
            tc.tile_pool(name="psy", bufs=2, space=bass.MemorySpace.PSUM) as psy,
        ):

            def emit_chunk(c0, W):
                sl = _mm_slices(W)
                xt = io.tile([P, ND, CWMAX], f32r, tag="xt", bufs=1)
                nc.sync.dma_start(xt[:, :, :W], xt_d[:, :, c0 : c0 + W])
                cb = io.tile([P, CWMAX], f32, tag="cb")
                nc.sync.dma_start(cb[:, :W], cb_d[:, c0 : c0 + W])
                gts = []

                # ---- phase 1: g = silu(x @ w1) * (x @ w3), transposed ----
                for hm in range(NH):
                    w1t = wts.tile([P, ND, P], f32r, tag="w1")
                    nc.sync.dma_start(w1t[:], w1_d[hm])
                    w3t = wts.tile([P, ND, P], f32r, tag="w3")
                    nc.sync.dma_start(w3t[:], w3_d[hm])
                    ph1 = psh.tile([P, CWMAX], f32, tag="h1")
                    ph3 = psh.tile([P, CWMAX], f32, tag="h3")
                    for dk in range(ND):
                        for s0, sw in sl:
                            nc.tensor.matmul(
                                ph1[:, s0 : s0 + sw],
                                w1t[:, dk, :],
                                xt[:, dk, s0 : s0 + sw],
                                start=(dk == 0),
                                stop=(dk == ND - 1),
                            )
                    if not MM_ONLY:
                        s1 = io.tile([P, CWMAX], f32, tag="s1")
                        nc.scalar.activation(s1[:, :W], ph1[:, :W], ACT.Sigmoid)
                        t1 = io.tile([P, CWMAX], f32, tag="t1")
                        nc.vector.tensor_mul(t1[:, :W], s1[:, :W], ph1[:, :W])
                    for dk in range(ND):
                        for s0, sw in sl:
                            nc.tensor.matmul(
                                ph3[:, s0 : s0 + sw],
                                w3t[:, dk, :],
                                xt[:, dk, s0 : s0 + sw],
                                start=(dk == 0),
                                stop=(dk == ND - 1),
                            )
                    g = big.tile([P, CWMAX], f32r, tag=f"g{hm}")
                    if not MM_ONLY:
                        nc.vector.tensor_mul(g[:, :W], t1[:, :W], ph3[:, :W])
                    else:
                        nc.sync.dma_start(g[:, :W], xt_d[:, 0, c0 : c0 + W])
                    gts.append(g)

                # ---- phase 2: yT = (g @ w2).T * combine ----
                for dm in range(ND):
                    d0 = dm * P
                    w2t = wts.tile([P, NH, P], f32r, tag="w2", bufs=2)
                    nc.sync.dma_start(w2t[:], w2_d[dm])
                    py = psy.tile([P, CWMAX], f32, tag="y")
                    for hk in range(NH):
                        for s0, sw in sl:
                            nc.tensor.matmul(
                                py[:, s0 : s0 + sw],
                                w2t[:, hk, :],
                                gts[hk][:, s0 : s0 + sw],
                                start=(hk == 0),
                                stop=(hk == NH - 1),
                            )
                    if not MM_ONLY or dm == 0:
                        y = io.tile([P, CWMAX], f32, tag="y_out")
                        nc.vector.tensor_mul(y[:, :W], py[:, :W], cb[:, :W])
                        nc.sync.dma_start(yt_d[d0 : d0 + P, c0 : c0 + W], y[:, :W])

            def emit_all():
                c0 = 0
                for W in chunks:
                    emit_chunk(c0, W)
                    c0 += W

            if BODY_REPEATS > 1:
                with tc.For_i(0, BODY_REPEATS, 1):
                    emit_all()
            else:
                emit_all()

    nc.compile()
    return nc


def _permute_inputs(xf_pad, w1e, w3e, w2e, C):
    """Pre-permute per-core tensors into the kernel's DMA-contiguous layouts."""
    # xt: [D, C] -> [P, ND, C]
    xt = xf_pad.reshape(ND, P, C).transpose(1, 0, 2).copy()
    # w1/w3: [D, H] -> [NH, P, ND, P_h]
    w1p = w1e.reshape(ND, P, NH, P).transpose(2, 1, 0, 3).copy()
    w3p = w3e.reshape(ND, P, NH, P).transpose(2, 1, 0, 3).copy()
    # w2: [H, D] -> [ND, P, NH, P_d]
    w2p = w2e.reshape(NH, P, ND, P).transpose(2, 1, 0, 3).copy()
    return xt, w1p, w3p, w2p


def _route(xf, gw):
    """Replicate the reference router in numpy fp32.

    Returns (top_idx [T,2], top_w [T,2]) matching jax.lax.top_k +
    softmax-over-the-2-selected-scores.
    """
    scores = xf @ gw.T                                   # [T, E] fp32
    order = np.argsort(-scores, axis=-1, kind="stable")  # ties -> lower idx
    top_idx = order[:, :TOPK]
    tw = np.take_along_axis(scores, top_idx, axis=-1).astype(np.float32)
    m = tw.max(axis=-1, keepdims=True)
    ex = np.exp(tw - m, dtype=np.float32)
    top_w = ex / ex.sum(axis=-1, keepdims=True)
    return top_idx, top_w


def make_in_maps(x, gate_w, w1, w2, w3):
    """Route + dispatch: returns (in_maps, sel, C, shape)."""
    x = np.asarray(x, dtype=np.float32)
    b, s, d = x.shape
    xf = np.ascontiguousarray(x.reshape(-1, d))
    gw = np.asarray(gate_w, dtype=np.float32)
    w1 = np.asarray(w1, dtype=np.float32)
    w2 = np.asarray(w2, dtype=np.float32)
    w3 = np.asarray(w3, dtype=np.float32)

    top_idx, top_w = _route(xf, gw)

    sel, cwt = [], []
    for ee in range(E):
        hit = top_idx == ee                      # [T, 2] bool
        rows = np.nonzero(hit.any(axis=1))[0]
        sel.append(rows)
        cwt.append(top_w[rows, hit[rows].argmax(axis=1)].astype(np.float32))

    cnt = max(len(r) for r in sel)
    C = ((max(cnt, 1) + CGRAN - 1) // CGRAN) * CGRAN

    in_maps = []
    for ee in range(E):
        rows, wts_e = sel[ee], cwt[ee]
        ne = len(rows)
        xtf = np.zeros((D, C), np.float32)
        xtf[:, :ne] = xf[rows].T
        cb_row = np.zeros((C,), np.float32)
        cb_row[:ne] = wts_e
        xt, w1p, w3p, w2p = _permute_inputs(xtf, w1[ee], w3[ee], w2[ee], C)
        if MM_DTYPE == "bf16":
            import ml_dtypes

            bf = ml_dtypes.bfloat16
            xt, w1p, w3p, w2p = (
                xt.astype(bf),
                w1p.astype(bf),
                w3p.astype(bf),
                w2p.astype(bf),
            )
        in_maps.append(
            {
                "xt": xt,
                "w1": w1p,
                "w3": w3p,
                "w2": w2p,
                "cb": np.ascontiguousarray(np.broadcast_to(cb_row, (P, C))),
            }
        )
    return in_maps, sel, C, (b, s, d)


def _pjrt_fn(nc, in_maps):
    """Build a reusable sharded-jit callable over device-resident inputs."""
    import jax
    from jax.experimental.shard_map import shard_map
    from jax.sharding import Mesh, NamedSharding, PartitionSpec

    from concourse import mybir
    from concourse.bass2jax import (
        _bass_exec_p,
        install_neuronx_cc_hook,
        partition_id_tensor,
    )

    install_neuronx_cc_hook()
    partition_name = nc.partition_id_tensor.name if nc.partition_id_tensor else None
    in_names, out_names, out_avals, zero_outs = [], [], [], []
    for alloc in nc.m.functions[0].allocations:
        if not isinstance(alloc, mybir.MemoryLocationSet):
            continue
        name = alloc.memorylocations[0].name
        if alloc.kind == "ExternalInput":
            if name != partition_name:
                in_names.append(name)
        elif alloc.kind == "ExternalOutput":
            out_names.append(name)
            shape = tuple(alloc.tensor_shape)
            dtype = mybir.dt.np(alloc.dtype)
            out_avals.append(jax.core.ShapedArray(shape, dtype))
            zero_outs.append(np.zeros(shape, dtype))
    n_params = len(in_names)
    all_in = list(in_names) + list(out_names)
    if partition_name is not None:
        all_in.append(partition_name)

    def _body(*args):
        operands = list(args)
        if partition_name is not None:
            operands.append(partition_id_tensor())
        return tuple(
            _bass_exec_p.bind(
                *operands,
                out_avals=tuple(out_avals),
                in_names=tuple(all_in),
                out_names=tuple(out_names),
                lowering_input_output_aliases=(),
                sim_require_finite=True,
                sim_require_nnan=True,
                nc=nc,
            )
        )

    mesh = Mesh(np.asarray(jax.devices()[:E]), ("core",))
    spec = PartitionSpec("core")
    fn = jax.jit(
        shard_map(
            _body,
            mesh=mesh,
            in_specs=(spec,) * (n_params + len(out_names)),
            out_specs=(spec,) * len(out_names),
            check_rep=False,
        ),
        keep_unused=True,
    )
    sh = NamedSharding(mesh, spec)
    dev_in = [
        jax.device_put(
            np.concatenate([np.asarray(in_maps[c][nm]) for c in range(E)], axis=0), sh
        )
        for nm in in_names
    ]
    for z in zero_outs:
        dev_in.append(jax.device_put(np.zeros((E * z.shape[0], *z.shape[1:]), z.dtype), sh))
    for a in dev_in:
        a.block_until_ready()
    return fn, dev_in


def measure_hw_ns(in_maps, C, r_big=41, iters=15):
    """Per-pass HW time via device-side For_i repeat-loop slope."""
    import time as _time

    import jax

    global BODY_REPEATS
    nc1 = _CACHE.get(C)
    if nc1 is None:
        nc1 = _build(C)
        _CACHE[C] = nc1
    old = BODY_REPEATS
    BODY_REPEATS = r_big
    try:
        ncR = _build(C)
    finally:
        BODY_REPEATS = old
    fn1, in1 = _pjrt_fn(nc1, in_maps)
    fnR, inR = _pjrt_fn(ncR, in_maps)
    for _ in range(2):
        jax.block_until_ready(fn1(*in1))
        jax.block_until_ready(fnR(*inR))
    t1s, tRs = [], []
    for _ in range(iters):
        t0 = _time.perf_counter()
        jax.block_until_ready(fn1(*in1))
        t1s.append(_time.perf_counter() - t0)
        t0 = _time.perf_counter()
        jax.block_until_ready(fnR(*inR))
        tRs.append(_time.perf_counter() - t0)
    return (min(tRs) - min(t1s)) * 1e9 / (r_big - 1)


def kernel(x, gate_w, w1, w2, w3):
    global LAST_RESULT
    in_maps, sel, C, (b, s, d) = make_in_maps(x, gate_w, w1, w2, w3)

    nc = _CACHE.get(C)
    if nc is None:
        nc = _build(C)
        _CACHE[C] = nc

    from concourse.bass_utils import run_bass_kernel_spmd

    res = run_bass_kernel_spmd(nc, in_maps, core_ids=list(range(E)), trace=TRACE)
    LAST_RESULT = res

    out = np.zeros((b * s, d), np.float32)
    for ee in range(E):
        ne = len(sel[ee])
        if ne:
            # yt is already combine-scaled on device; rows are unique per
            # expert so fancy-index += is safe.
            out[sel[ee]] += res.results[ee]["yt"][:, :ne].T
    return out.reshape(b, s, d)
